# revision 47
# baseline (speedup 1.0000x reference)
"""Causal multi-head self-attention with RoPE on 8 Trainium2 NeuronCores.

Sharding: data-parallel over batch (B=4 -> 2 cores per batch) x tensor-parallel
over heads (16 heads -> 8 per core). Each core computes q/k/v projections for
its 8 heads, RoPE, causal attention, and a partial o_proj; the host sums the
two partial o_proj outputs per batch.

Structure (v5 = v3 pipeline + host tables + faster startup/tail):
  - One fused pipeline: per 512-query chunk sc, the Q/K/V projection chains of
    chunk sc+1 and the o_proj chains of chunk sc-1 are interleaved as PE
    "filler" work inside attention(sc)'s scores/AV stream, so the tensor
    engine never idles at phase boundaries and stays at full p-state clock.
    Fillers are front-loaded at sc=0 (covers the RoPE(0) wait) and reserved
    at each chunk's end (covers the normalization drain).
  - bf16 everywhere on the PE (1 cyc/row at any N). The tile_position-packed
    score pairs overlap in the PE row groups (2nd matmul of a pair is ~free).
  - RoPE swap is a PE matmul against a bf16 block-swap permutation; its
    emission is deferred one filler so the PE never waits on the PSUM->SBUF
    cast of its input.
  - Rotary cos/sin tables and the swap permutation are precomputed on HOST
    (float64) and DMA'd in: no device table build, and the scalar engine's
    activation table holds only Exp (one warmup exp at t0 makes it
    resident). Startup DMAs are ordered x0/wq first (interleaved per dc on
    sync/scalar) with tables/wk on gpsimd, so chain 0 starts ~14us in.
  - PSUM budget (8 banks): score pairs [128,1024] x2 (4 banks), work tiles
    [128,512] x2 (proj chains / RoPE swap / o_proj, 2 banks), AV
    accumulators [65,512] x2 (2 banks, drained to SBUF by DVE immediately).
  - exp runs on the scalar engine straight out of PSUM over the packed
    head-pair tile, one instruction per contiguous region -- a merged
    2-level-AP exp measures ~45% slower on hw, so diagonal tiles keep two.
    Causal masking of the diagonal 128-col block is a gpsimd affine_select.
  - Normalization: DVE reciprocal (via a partition-0 staging copy -- the
    approx-reciprocal sequence cannot take a partition-offset input),
    gpsimd broadcast, DVE multiply writing heads_t directly for the even
    head and via one DMA for the odd head (engines cannot shift partitions;
    gpsimd has no PSUM port). The LAST block skips that DMA: the epilogue
    splits each dc3 o_proj matmul into two K=64 halves, consuming the odd
    head's hn straight from partitions 0-63 against a duplicate wo row-set,
    so the program tail never waits on a cross-partition move.
"""

import sys

sys.path.insert(0, "/opt/trn_rl_repo")

import numpy as np

import concourse.bass as bass
import concourse.tile as tile
from concourse import bacc, mybir
from concourse.bass_utils import run_bass_kernel_spmd

B, S, D, H = 4, 2048, 1024, 16
DK = D // H            # 64
HPC = H // 2           # 8 heads per core
DPC = HPC * DK         # 512 head dims per core
N_CORES = 8
HALF = DK // 2         # 32 rotary pairs
THETA = 10000.0
NKC = S // 128         # 16 key tiles
NSC = S // 512         # 4 query/proj chunks

AF = mybir.ActivationFunctionType
F32 = mybir.dt.float32
F32R = mybir.dt.float32r
BF16 = mybir.dt.bfloat16
I32 = mybir.dt.int32

TWO_PI = 2.0 * np.pi
# 3-term Cody-Waite split of 2*pi (c1/c2 have short mantissas so k*c is exact)
_CW_C1 = 6.28125
_CW_C2 = float(np.float32(9.67025756835937500e-4))
_CW_C3 = float(TWO_PI - _CW_C1 - np.float32(9.67025756835937500e-4))


def _build_program(debug=False):
    nc = bacc.Bacc("TRN2", target_bir_lowering=False, debug=False)

    xT = nc.dram_tensor("xT", [D, S], BF16, kind="ExternalInput").ap()
    wqT = nc.dram_tensor("wqT", [D, DPC], BF16, kind="ExternalInput").ap()
    wkT = nc.dram_tensor("wkT", [D, DPC], BF16, kind="ExternalInput").ap()
    wvT = nc.dram_tensor("wvT", [D, DPC], BF16, kind="ExternalInput").ap()
    woT = nc.dram_tensor("woT", [DPC, D], BF16, kind="ExternalInput").ap()
    cbig_in = nc.dram_tensor("cbig", [128, S], BF16, kind="ExternalInput").ap()
    sbig_in = nc.dram_tensor("sbig", [128, S], BF16, kind="ExternalInput").ap()
    pswap_in = nc.dram_tensor("pswap", [128, 128], BF16, kind="ExternalInput").ap()
    y = nc.dram_tensor("y", [S, D], F32, kind="ExternalOutput").ap()

    dbg = None
    if debug:
        dbg = {
            "cs_dump": nc.dram_tensor("cs_dump", [2, 128, S], BF16, kind="ExternalOutput").ap(),
            "qk_dump": nc.dram_tensor("qk_dump", [128, 8, S], BF16, kind="ExternalOutput").ap(),
            "vp_dump": nc.dram_tensor("vp_dump", [128, NKC, HPC * (DK + 1)], BF16, kind="ExternalOutput").ap(),
            "heads_dump": nc.dram_tensor("heads_dump", [128, 4, S], BF16, kind="ExternalOutput").ap(),
            "recip_dump": nc.dram_tensor("recip_dump", [NSC, 2, 512], F32, kind="ExternalOutput").ap(),
            "rb_dump": nc.dram_tensor("rb_dump", [NSC, 2, DK, 512], F32, kind="ExternalOutput").ap(),
        }

    with tile.TileContext(nc) as tc:
        _emit(nc, tc, xT, wqT, wkT, wvT, woT, cbig_in, sbig_in, pswap_in, y, dbg)

    nc.compile()
    return nc


def _emit(nc, tc, xT, wqT, wkT, wvT, woT, cbig_in, sbig_in, pswap_in, y, dbg=None):
    import contextlib

    ctx = contextlib.ExitStack()
    with ctx:
        persist = ctx.enter_context(tc.tile_pool(name="persist", bufs=1))
        ones_col = persist.tile([128, 1], BF16)
        nc.vector.memset(ones_col, 1.0)
        # warmup exp: makes the Exp activation table resident before the
        # attention stream needs it (the load is ~1.3us on the idle queue)
        wsrc = persist.tile([1, 8], F32, name="wsrc")
        nc.vector.memset(wsrc, 0.0)
        wdst = persist.tile([1, 8], BF16, name="wdst")
        nc.scalar.activation(wdst, wsrc, AF.Exp, scale=1.0)
        # P_swap: swap 32-row blocks within each 64-block (host-precomputed);
        # loaded on the gpsimd queue so it never delays the x0/wq stream that
        # feeds the first proj chain (it's only needed by the first RoPE tail)
        p_swap = persist.tile([128, 128], BF16)
        nc.gpsimd.dma_start(out=p_swap, in_=pswap_in)

        # cbig/sbig [128, S] bf16: 32-row blocks [cos;cos] and [-sin;sin],
        # replicated to rows 64-127, so RoPE on a [128, s] slice of Q^T/K^T is
        #   q' = q * cbig + (P_swap @ q) * sbig
        cs_pool = ctx.enter_context(tc.tile_pool(name="cs", bufs=1))
        cbig = cs_pool.tile([128, S], BF16)
        sbig = cs_pool.tile([128, S], BF16)

        # ---------------- persistent data tiles ----------------
        qkT_pool = ctx.enter_context(tc.tile_pool(name="qkT", bufs=1))
        qkT = qkT_pool.tile([128, 8, S], BF16)       # q units 0-3, k units 4-7
        vp_pool = ctx.enter_context(tc.tile_pool(name="vp", bufs=1))
        vp = vp_pool.tile([128, NKC, HPC * (DK + 1)], BF16)
        vp_heads = vp.rearrange("p s (h c) -> p s h c", h=HPC)
        nc.scalar.copy(vp_heads[:, :, :, DK:DK + 1],
                       ones_col.to_broadcast((128, NKC, HPC, 1)))

        w_pool = ctx.enter_context(tc.tile_pool(name="w", bufs=1))
        w_qk = w_pool.tile([128, 2, D // 128, DPC], BF16)
        wv_t = w_pool.tile([128, D // 128, DPC], BF16)

        xts_pool = ctx.enter_context(tc.tile_pool(name="xts", bufs=3))
        rope_pool = ctx.enter_context(tc.tile_pool(name="rope", bufs=4))
        e_pool = ctx.enter_context(tc.tile_pool(name="expp", bufs=6))

        # PSUM: 16 KiB/partition = 8 banks, fully budgeted:
        #   sc: [128,1024] f32 x2  (4 banks) score pairs
        #   wk: [128,512]  f32 x2  (2 banks) proj chains / RoPE swap / o_proj
        #   o:  [65,512]   f32 x2  (2 banks) AV accumulators
        ps = ctx.enter_context(tc.tile_pool(name="ps", bufs=1, space="PSUM"))

        # ---------------- table/weight/x DMAs (tables from host) -----------
        # the first proj chain's deps (x chunk 0 + wq, interleaved per dc)
        # lead the sync/scalar queues so chain 0 starts ~5us in; the rotary
        # tables (needed only by the first RoPE tail) and the K/V weights
        # ride the gpsimd queue in parallel
        nc.gpsimd.dma_start(out=cbig[:, 0:1024], in_=cbig_in[:, 0:1024])
        nc.gpsimd.dma_start(out=sbig[:, 0:1024], in_=sbig_in[:, 0:1024])
        nc.gpsimd.dma_start(out=cbig[:, 1024:S], in_=cbig_in[:, 1024:S])
        nc.gpsimd.dma_start(out=sbig[:, 1024:S], in_=sbig_in[:, 1024:S])
        xts0 = xts_pool.tile([128, D // 128, 512], BF16, name="xts")
        for dc in range(D // 128):
            eng = nc.sync if (dc % 2 == 0) else nc.scalar
            eng.dma_start(out=xts0[:, dc, :],
                          in_=xT[dc * 128:(dc + 1) * 128, 0:512])
            eng.dma_start(out=w_qk[:, 0, dc, :],
                          in_=wqT[dc * 128:(dc + 1) * 128, :])
        xts_tiles = {0: xts0}
        for dc in range(D // 128):
            eng = nc.gpsimd if (dc % 2 == 0) else nc.scalar
            eng.dma_start(out=w_qk[:, 1, dc, :],
                          in_=wkT[dc * 128:(dc + 1) * 128, :])
        for dc in range(D // 128):
            eng = nc.scalar if (dc % 2 == 0) else nc.sync
            eng.dma_start(out=wv_t[:, dc, :],
                          in_=wvT[dc * 128:(dc + 1) * 128, :])

        # pools whose SBUF space reuses the (released) table scratch
        heads_pool = ctx.enter_context(tc.tile_pool(name="heads", bufs=1))
        # one tile per head-pair: keeps o_proj reads dependent only on the
        # pairs they actually consume (a single [128,4,S] tile made every
        # o_proj chain wait for the newest normalization write)
        heads_hp = [heads_pool.tile([128, S], BF16, name=f"h{i}")
                    for i in range(DPC // 128)]
        wo_pool = ctx.enter_context(tc.tile_pool(name="wo", bufs=1))
        wo_t = wo_pool.tile([128, DPC // 128, D], BF16)
        for dc in range(DPC // 128):
            eng = nc.sync if (dc % 2 == 0) else nc.scalar
            eng.dma_start(out=wo_t[:, dc, :],
                          in_=woT[dc * 128:(dc + 1) * 128, :])
        # last head's wo rows again at partitions 0-63: the epilogue contracts
        # the final block's un-DMA'd odd head (hn, partitions 0-63) against
        # these as a K=64 matmul
        wo_b = wo_pool.tile([DK, D], BF16, name="wo_b")
        nc.gpsimd.dma_start(out=wo_b, in_=woT[DPC - DK:DPC, :])
        norm_pool = ctx.enter_context(tc.tile_pool(name="norm", bufs=3))
        y_pool = ctx.enter_context(tc.tile_pool(name="yout", bufs=2))

        # ---------------- emission helpers ----------------
        deferred = []           # RoPE tails, emitted one filler late so the
                                # swap matmul never stalls on the qt_sb cast

        def flush_deferred(n=1):
            for _ in range(min(n, len(deferred))):
                deferred.pop(0)()

        def emit_x_load(sc):
            xts_t = xts_pool.tile([128, D // 128, 512], BF16, name="xts")
            for dc in range(D // 128):
                nc.sync.dma_start(
                    out=xts_t[:, dc, :],
                    in_=xT[dc * 128:(dc + 1) * 128, bass.ts(sc, 512)])
            xts_tiles[sc] = xts_t

        def proj_unit(sc, qk_idx, et):
            # one 128-dim tile of the Q or K projection for chunk sc, + RoPE
            ssl = bass.ts(sc, 512)
            xts_t = xts_tiles[sc]
            pt = ps.tile([128, 512], F32, name="wk", bufs=2)
            for dc in range(D // 128):
                nc.tensor.matmul(pt, w_qk[:, qk_idx, dc, bass.ts(et, 128)],
                                 xts_t[:, dc, :],
                                 start=(dc == 0), stop=(dc == D // 128 - 1))
            qt_sb = rope_pool.tile([128, 512], BF16, name="qt_sb")
            nc.vector.tensor_copy(qt_sb, pt)

            def tail(qt_sb=qt_sb, ssl=ssl, u=qk_idx * 4 + et):
                sw = ps.tile([128, 512], F32, name="wk", bufs=2)
                nc.tensor.matmul(sw, p_swap, qt_sb, start=True, stop=True)
                g1 = rope_pool.tile([128, 512], BF16, name="g1")
                nc.vector.tensor_mul(g1, qt_sb, cbig[:, ssl])
                d1 = rope_pool.tile([128, 512], BF16, name="d1")
                nc.vector.tensor_mul(d1, sw, sbig[:, ssl])
                nc.vector.tensor_add(qkT[:, u, ssl], g1, d1)

            deferred.append(tail)
            if len(deferred) > 1:
                flush_deferred(1)

        def v_unit(sc, st4):
            xts_t = xts_tiles[sc]
            pv = ps.tile([128, 512], F32, name="wk", bufs=2)
            for dc in range(D // 128):
                nc.tensor.matmul(pv, xts_t[:, dc, bass.ts(st4, 128)],
                                 wv_t[:, dc, :],
                                 start=(dc == 0), stop=(dc == D // 128 - 1))
            nc.vector.tensor_copy(vp_heads[:, sc * 4 + st4, :, 0:DK],
                                  pv.rearrange("p (h c) -> p h c", h=HPC))
            flush_deferred(1)

        ya_tiles = []
        final_hn = []

        def epi_partial(st4, nb):
            # dc0-2 of the last chunk's o_proj, banked to SBUF
            st = (NSC - 1) * 4 + st4
            py = ps.tile([128, 512], F32, name="wk", bufs=2)
            for dc in range(DPC // 128 - 1):
                nc.tensor.matmul(py, heads_hp[dc][:, bass.ts(st, 128)],
                                 wo_t[:, dc, bass.ts(nb, 512)],
                                 start=(dc == 0), stop=(dc == DPC // 128 - 2))
            ya = y_pool.tile([128, 512], BF16, name="ya", bufs=8)
            nc.vector.tensor_copy(ya, py)
            ya_tiles.append(ya)
            flush_deferred(1)

        def o_chain(qc, st4, nb):
            st = qc * 4 + st4
            py = ps.tile([128, 512], F32, name="wk", bufs=2)
            for dc in range(DPC // 128):
                nc.tensor.matmul(py, heads_hp[dc][:, bass.ts(st, 128)],
                                 wo_t[:, dc, bass.ts(nb, 512)],
                                 start=(dc == 0), stop=(dc == DPC // 128 - 1))
            y_sb = y_pool.tile([128, 512], F32, name="y_sb")
            nc.vector.tensor_copy(y_sb, py)
            eng = nc.sync if (st4 % 2 == 0) else nc.gpsimd
            eng.dma_start(out=y[st * 128:(st + 1) * 128, bass.ts(nb, 512)],
                          in_=y_sb)
            flush_deferred(1)

        def attention(qc, front_f, loop_f, tail_f, late_f=()):
            n_kt = 4 * qc + 4
            # front_f run before the first score pair (covers the RoPE wait at
            # qc=0); loop_f are slotted between pairs at a pace that never
            # starves the exp pipeline (~1 chain per 5 pairs); tail_f are
            # heads_t-independent and run during the final norm drain.
            for f in front_f:
                f()
            pair_total = 4 * n_kt
            stride = 5 if len(loop_f) * 5 <= pair_total else max(
                1, pair_total // max(1, len(loop_f)))
            state = {"pair": 0, "fi": 0}

            def maybe_filler():
                if (state["fi"] < len(loop_f)
                        and state["pair"] >= (state["fi"] + 1) * stride):
                    loop_f[state["fi"]]()
                    state["fi"] += 1
                state["pair"] += 1

            late = {"fi": 0, "pair": 0}

            def maybe_late():
                # late fillers: only legal inside the last head-pair block
                # (they read this chunk's earlier head pairs)
                if (late["fi"] < len(late_f)
                        and late["pair"] >= (late["fi"] + 1) * 2):
                    late_f[late["fi"]]()
                    late["fi"] += 1
                late["pair"] += 1

            for hp in range(HPC // 2):
                hA, hB = 2 * hp, 2 * hp + 1
                o_ts = [ps.tile([DK + 1, 512], F32, name="o", bufs=2)
                        for _ in range(2)]

                def emit_scores(kt):
                    diag = (kt // 4 == qc)
                    co = 128 * (kt % 4) if diag else 0
                    n = 512 - co
                    ktsl = bass.ts(kt, 128)
                    qsl = bass.ds(qc * 512 + co, n)
                    sc_t = ps.tile([128, 1024], F32, name="sc", bufs=2)
                    for i, (ro, tp) in enumerate(((0, (0, 0)), (64, (64, 0)))):
                        nc.tensor.matmul(
                            sc_t[:, i * 512:i * 512 + n],
                            qkT[ro:ro + 64, 4 + hp, ktsl],
                            qkT[ro:ro + 64, hp, qsl],
                            start=True, stop=True, tile_position=tp)
                    e_t = e_pool.tile([128, 1024], BF16, name="e_t")
                    if co == 0:
                        nc.scalar.activation(e_t, sc_t, AF.Exp,
                                             scale=float(1.0 / np.sqrt(DK)))
                    else:
                        # NOTE: a single 2-level-AP exp over both regions is
                        # ~45% slower on hw (slow AGU path) -- keep 2 instrs
                        for i in range(2):
                            nc.scalar.activation(
                                e_t[:, i * 512:i * 512 + n],
                                sc_t[:, i * 512:i * 512 + n], AF.Exp,
                                scale=float(1.0 / np.sqrt(DK)))
                    if diag:
                        for i in range(2):
                            nc.gpsimd.affine_select(
                                out=e_t[:, i * 512:i * 512 + 128],
                                in_=e_t[:, i * 512:i * 512 + 128],
                                pattern=[[1, 128]], base=0, channel_multiplier=-1,
                                compare_op=mybir.AluOpType.is_ge, fill=0.0)
                    return e_t, co, n

                def emit_av(kt, e_t, co, n):
                    for i, h in enumerate((hA, hB)):
                        nc.tensor.matmul(
                            o_ts[i][:, co:512],
                            vp[:, kt, h * (DK + 1):(h + 1) * (DK + 1)],
                            e_t[:, i * 512:i * 512 + n],
                            start=(kt == 0), stop=(kt == n_kt - 1))

                last_hp = (hp == HPC // 2 - 1)
                maybe_filler()
                pend = emit_scores(0)
                for kt in range(1, n_kt):
                    maybe_filler()
                    if last_hp:
                        maybe_late()
                    e = emit_scores(kt)
                    emit_av(kt - 1, *pend)
                    pend = e
                emit_av(n_kt - 1, *pend)
                maybe_filler()
                while last_hp and late["fi"] < len(late_f):
                    late_f[late["fi"]]()
                    late["fi"] += 1

                # drain accumulators to SBUF fast (frees the PSUM "o" slots),
                # then normalize by the ones-column denominator. The very last
                # block skips the staging copy and reads PSUM directly -- its
                # slots need no recycling and the drain is the program tail.
                last_blk = (qc == NSC - 1 and hp == HPC // 2 - 1)
                # at a CHUNK boundary (hp3) the vector queue is the
                # serializer: the next chunk's filler chains wait on it for
                # their PSUM-releasing casts while the scalar engine is idle
                # (no scores to exp yet) -- so hp3's staging copies go to the
                # scalar engine instead (it has a PSUM read port)
                on_scalar = (hp == HPC // 2 - 1)

                def stage_copy(out_t, in_t):
                    if on_scalar:
                        nc.scalar.copy(out_t, in_t)
                    else:
                        nc.vector.tensor_copy(out_t, in_t)

                obs = []
                for i in range(2):
                    if last_blk:
                        obs.append(o_ts[i])
                    else:
                        ob = norm_pool.tile([DK + 1, 512], F32, name="ob")
                        stage_copy(ob, o_ts[i])
                        obs.append(ob)
                rbs = []
                for i in range(2):
                    dsb = norm_pool.tile([1, 512], F32, name="dsb")
                    stage_copy(dsb, obs[i][DK:DK + 1, :])
                    recip = norm_pool.tile([1, 512], F32, name="recip")
                    nc.vector.reciprocal_approx_fast(recip, dsb)
                    rb = norm_pool.tile([DK, 512], F32, name="rb")
                    nc.gpsimd.partition_broadcast(rb, recip)
                    rbs.append(rb)
                    if dbg is not None and hp == 0:
                        nc.sync.dma_start(out=dbg["recip_dump"][qc, i].unsqueeze(0), in_=recip)
                        nc.sync.dma_start(out=dbg["rb_dump"][qc, i], in_=rb)
                nc.vector.tensor_mul(
                    heads_hp[hp][0:DK, bass.ts(qc, 512)], obs[0][0:DK, :], rbs[0])
                hn = norm_pool.tile([DK, 512], BF16, name="hn")
                nc.vector.tensor_mul(hn, obs[1][0:DK, :], rbs[1])
                if last_blk:
                    # the epilogue consumes hn straight from partitions 0-63
                    # (split K=64 matmuls) -- no cross-partition DMA on the
                    # program tail
                    final_hn.append(hn)
                else:
                    # gpsimd queue: keeps this DMA's completion semaphore off
                    # the sync queue, whose batched sems otherwise serialize
                    # the next block's norm behind it (~3us all-idle bubble)
                    nc.gpsimd.dma_start(
                        out=heads_hp[hp][DK:128, bass.ts(qc, 512)], in_=hn)

            for f in tail_f:
                f()

        # ---------------- fused schedule ----------------
        def chunk_fillers(sc):
            out = []
            for qk_idx in (0, 1):
                for et in range(4):
                    out.append(lambda s=sc, q=qk_idx, e=et: proj_unit(s, q, e))
            for st4 in range(4):
                out.append(lambda s=sc, t=st4: v_unit(s, t))
            return out

        def oproj_fillers(qc):
            return [lambda q=qc, t=st4, n=nb: o_chain(q, t, n)
                    for st4 in range(4) for nb in range(2)]

        for f in chunk_fillers(0):   # prologue: chunk-0 projections straight
            f()
        for sc in range(NSC):
            front_f, loop_f, tail_f = [], [], []
            if sc + 1 < NSC:
                emit_x_load(sc + 1)
                proj_f = chunk_fillers(sc + 1)
                nfront = 5 if sc == 0 else 1
                front_f = proj_f[:nfront]
                proj_f = proj_f[nfront:]
                loop_f += proj_f[:len(proj_f) - 2]
                tail_f += proj_f[len(proj_f) - 2:]
            if sc == NSC - 1:
                # all deferred o_proj chains land in the last window -- it is
                # the most exp-bound one, with enough PE slack to hide them.
                # The last chunk's own o_proj dc0-2 partials run only inside
                # the hp3 stretch (late_f) -- after the hp0-2 norms they read
                # are emitted -- right before the final normalization.
                for qcp in range(NSC - 1):
                    loop_f += oproj_fillers(qcp)
                late_f = [lambda t=st4, n=nb: epi_partial(t, n)
                          for st4 in range(4) for nb in range(2)]
                attention(sc, front_f, loop_f, tail_f, late_f)
            else:
                attention(sc, front_f, loop_f, tail_f)
        flush_deferred(99)
        # epilogue tail: one dc3 matmul per chain (the only true dependent of
        # the final pair's normalization) + recombine with the banked dc0-2
        # partials. The partial chains themselves ran as hp3-stretch fillers
        # inside attention(3) -- emitted before the final norm so the batched
        # event-semaphores don't serialize them behind it.
        for k, (st4, nb) in enumerate(
                [(t, n) for t in range(4) for n in range(2)]):
            st = (NSC - 1) * 4 + st4
            dc = DPC // 128 - 1
            pyb = ps.tile([128, 512], F32, name="wk", bufs=2)
            nc.tensor.matmul(pyb, heads_hp[dc][0:DK, bass.ts(st, 128)],
                             wo_t[0:DK, dc, bass.ts(nb, 512)],
                             start=True, stop=False)
            nc.tensor.matmul(pyb, final_hn[0][:, bass.ts(st4, 128)],
                             wo_b[:, bass.ts(nb, 512)],
                             start=False, stop=True)
            y_sb = y_pool.tile([128, 512], F32, name="y_sb")
            nc.vector.tensor_add(y_sb, ya_tiles[k], pyb)
            eng = (nc.sync, nc.gpsimd, nc.scalar)[k % 3]
            eng.dma_start(out=y[st * 128:(st + 1) * 128, bass.ts(nb, 512)],
                          in_=y_sb)

        if dbg is not None:
            nc.sync.dma_start(out=dbg["cs_dump"][0], in_=cbig)
            nc.sync.dma_start(out=dbg["cs_dump"][1], in_=sbig)
            nc.sync.dma_start(out=dbg["qk_dump"], in_=qkT)
            nc.sync.dma_start(out=dbg["vp_dump"], in_=vp)
            for i in range(4):
                nc.sync.dma_start(out=dbg["heads_dump"][:, i, :], in_=heads_hp[i])


def _host_tables(pos_np):
    import ml_dtypes
    inv = 1.0 / (THETA ** (np.arange(HALF, dtype=np.float64) * 2.0 / DK))
    ang = pos_np.astype(np.float64)[None, :] * inv[:, None]      # [32, S]
    c, s = np.cos(ang), np.sin(ang)
    cs64 = np.concatenate([c, c], axis=0)                        # [64, S]
    ss64 = np.concatenate([-s, s], axis=0)
    cbig = np.tile(cs64, (2, 1)).astype(ml_dtypes.bfloat16)      # [128, S]
    sbig = np.tile(ss64, (2, 1)).astype(ml_dtypes.bfloat16)
    pswap = np.zeros((128, 128), dtype=np.float32)
    pswap[np.arange(128), np.arange(128) ^ 32] = 1.0
    return cbig, sbig, pswap.astype(ml_dtypes.bfloat16)


_program_cache = None


def _get_program():
    global _program_cache
    if _program_cache is None:
        _program_cache = _build_program()
    return _program_cache


# dk permutation: evens then odds within each head's 64 dims
_PERM64 = np.concatenate([np.arange(0, DK, 2), np.arange(1, DK, 2)])


def _make_in_maps(x, Wq, Wk, Wv, Wo, pos_np):
    import ml_dtypes
    cbig_np, sbig_np, pswap_np = _host_tables(pos_np)
    in_maps = []
    for c in range(N_CORES):
        b, hg = c // 2, c % 2
        rows = hg * DPC + np.concatenate(
            [h * DK + _PERM64 for h in range(HPC)])
        in_maps.append({
            "xT": np.ascontiguousarray(x[b].T).astype(ml_dtypes.bfloat16),
            "wqT": np.ascontiguousarray(Wq[rows, :].T).astype(ml_dtypes.bfloat16),
            "wkT": np.ascontiguousarray(Wk[rows, :].T).astype(ml_dtypes.bfloat16),
            "wvT": np.ascontiguousarray(
                Wv[hg * DPC:(hg + 1) * DPC, :].T).astype(ml_dtypes.bfloat16),
            "woT": np.ascontiguousarray(
                Wo[:, hg * DPC:(hg + 1) * DPC].T).astype(ml_dtypes.bfloat16),
            "cbig": cbig_np,
            "sbig": sbig_np,
            "pswap": pswap_np,
        })
    return in_maps


def kernel(x, Wq, Wk, Wv, Wo, token_positions):
    x = np.asarray(x, dtype=np.float32)
    Wq = np.asarray(Wq, dtype=np.float32)
    Wk = np.asarray(Wk, dtype=np.float32)
    Wv = np.asarray(Wv, dtype=np.float32)
    Wo = np.asarray(Wo, dtype=np.float32)
    pos_np = np.ascontiguousarray(np.asarray(token_positions, dtype=np.int32))

    nc = _get_program()
    in_maps = _make_in_maps(x, Wq, Wk, Wv, Wo, pos_np)
    res = run_bass_kernel_spmd(nc, in_maps, list(range(N_CORES)))
    out = np.empty((B, S, D), dtype=np.float32)
    for b in range(B):
        out[b] = res.results[2 * b]["y"] + res.results[2 * b + 1]["y"]
    return out

